# revision 44
# baseline (speedup 1.0000x reference)
"""Bayesian linear layer on 8 Trainium2 NeuronCores.

Computes: weight = mu + softplus(rho) * eps  (elementwise, [O, I])
          bias   = b_mu + softplus(b_rho) * b_eps              ([O])
          y      = x @ weight.T + bias       ([N, I] @ [I, O] -> [N, O])

Shapes: x [8192, 4096], weight_* [16384, 4096], bias_* [16384].

Sharding: column-parallel over 8 cores - each core owns 2048 output
features (its slice of the weight/bias params), x is replicated. Each
core computes an independent [8192, 2048] output slice; the host
concatenates along the feature dim. No collectives needed.

Schedule notes. The kernel is PE-bound: 8192 bf16 matmuls of
[128,128]x[128,512] stream at ~220 ns each -> ~1.81 ms floor (measured
1.886 ms, PE ~93% busy). fp8-e4m3 DoubleRow measures the same ns/col
on this hw (2x MACs/cycle, not the cost model's 4x), so the 3-term
hi/lo fp8 split needed to pass the 2e-2 gate would cost 1.5x bf16 -
bf16 single pass is the fastest correct schedule. What keeps the PE
fed (each item fixed a measured stall class):
 - x host-repacked to [128, mc, kt, 256]: each m-chunk's load is one
   16 KB contiguous slab per partition (512-byte gathers ran the DMA
   engines at ~19 GB/s and starved the PE at chunk boundaries).
 - The gpsimd queue issues ONLY x-slab loads (pair-0's 32 up front;
   the pool-rotation WAR sems self-pace it XLOOK ahead). Pair-1 loads
   are emitted inside the pair-1 loop - emitting them earlier would
   deadlock: their buffer-WAR waits on pair-1 matmul progress would
   sit ahead of the pair-1 weight materialization in queue order.
 - The vector queue runs ONLY PSUM drains (+bias add) and nothing
   that can wait on a DMA: in-order queues transmit any materialize
   stall into late drains -> PSUM WAR -> PE stall (the dominant v2/v4
   failure mode).
 - Pair-1 weight materialization is split: the batched param DMA (one
   3 KB-row transfer, [rho|eps|mu]) issues on sync at m-chunk mc, its
   softplus (scalar exp/ln + vector mul/add) runs at mc+1, so the
   vector ops find their inputs resident.
 - y stores issue from sync; weights stay resident in SBUF across both
   pair passes (x read twice; blocks 2,3 materialize during pair 0).
 - Residual ~75 us: on-demand 16 KB instruction-page fetches (~1 us
   every ~25 us of tensor-queue program) and the startup chase (first
   block's softplus is scalar-engine-bound at ~46 us vs 14 us of PE
   demand).
"""

import numpy as np
import ml_dtypes

import concourse.bass as bass
import concourse.mybir as mybir
import concourse.tile as tile
from concourse.bass_utils import run_bass_kernel_spmd
from concourse.vector_clock import ScopedClock, VectorClock

N_CORES = 8
N_TOK = 8192
IN_F = 4096
OUT_F = 16384
O_PER = OUT_F // N_CORES  # 2048 out features per core

P = 128
KT = IN_F // P       # 32 k-tiles
OC = 512             # o-chunk: matmul moving width / PSUM bank
NOC = O_PER // OC    # 4 o-chunks

M_CHUNK = 256            # tokens per x slab (2 lhsT subtiles of 128)
MC = N_TOK // M_CHUNK    # 32 m-chunks
MSUB = M_CHUNK // P      # 2
OCS = 512                # stage chunk for weight materialization
XLOOK = 2                # x slabs enqueued ahead of consumption

F32 = mybir.dt.float32
BF16 = mybir.dt.bfloat16
AF = mybir.ActivationFunctionType
ALU = mybir.AluOpType


def _patch_tile_drain():
    """The walrus build here caps sync-wait commands per CTRL_NO_STRUCT
    instruction; Tile's kernel-tail Drain overflows it. Spread the waits
    across nop carriers (one wait each) before the drain."""
    if getattr(tile.TileContext, "_drain_patched", False):
        return

    def _drain_and_barrier(self, tick_clock, wait_clock):
        nc = self.nc
        gc = tick_clock.global_clock
        n = len(gc)
        for i in range(n):
            t = gc[i]
            if t > 0:
                sub = [0] * n
                sub[i] = t
                carrier = nc.sync.nop(nofuse=True)
                wait_clock.add_sem_waits(
                    carrier.ins, ScopedClock({None: VectorClock(sub)})
                )
        nc.sync.drain()
        nc.all_engine_barrier()
        popped = nc._tile_sem_poison_stack.pop()
        assert popped is self._sem_poison
        nc.clear_and_free_semaphores(list(self.sems.allocated().values()))
        nc.all_engine_barrier()

    tile.TileContext._drain_and_barrier = _drain_and_barrier
    tile.TileContext._drain_patched = True


def _split_sync_waits(nc, max_waits=1):
    """This container's walrus build accepts at most ONE sync-wait command
    per instruction (a 2-wait TensorTensor fails codegen with 'Too many
    sync wait commands'). Tile emits up to 3. Spill the excess onto
    same-engine InstNoOp carriers inserted immediately before the
    overloaded instruction - same-engine program order preserves the
    wait-before-execute semantics."""
    n_spilled = 0
    for fn in nc.m.functions:
        for bb in fn.blocks:
            insts = list(bb.instructions)
            out = []
            changed = False
            for inst in insts:
                si = inst.sync_info
                if si is not None and si.on_wait and len(si.on_wait) > max_waits:
                    waits = list(si.on_wait)
                    spill, keep = waits[:-max_waits], waits[-max_waits:]
                    for w in spill:
                        nop = mybir.InstNoOp(
                            name=f"I-waitspill-{nc.next_id()}", ins=[], outs=[]
                        )
                        nop.engine = inst.engine
                        nop.sync_info = mybir.SyncInfo(on_wait=[w], on_update=[])
                        out.append(nop)
                        n_spilled += 1
                    inst.sync_info = mybir.SyncInfo(
                        on_wait=keep, on_update=list(si.on_update)
                    )
                    changed = True
                out.append(inst)
            if changed:
                bb.instructions = out
    return n_spilled


def _build():
    _patch_tile_drain()
    nc = bass.Bass()

    # x: [p, mc, kt, n] so each (mc) slab is 16 KB contiguous per partition
    xA = nc.dram_tensor("xA", [P, MC, KT, M_CHUNK], BF16, kind="ExternalInput")
    # w params batched per stage set: [k, h, q, p, rho|eps|mu] - one 3 KB-row
    # DMA materializes a whole [128, OCS] weight chunk (3 issues -> 1)
    wprm = nc.dram_tensor(
        "wprm", [KT, 2, 2, P, 3 * OCS], BF16, kind="ExternalInput"
    )
    bmu = nc.dram_tensor("bmu", [1, O_PER], BF16, kind="ExternalInput")
    brho = nc.dram_tensor("brho", [1, O_PER], BF16, kind="ExternalInput")
    beps = nc.dram_tensor("beps", [1, O_PER], BF16, kind="ExternalInput")
    y = nc.dram_tensor("y", [N_TOK, O_PER], F32, kind="ExternalOutput")

    with tile.TileContext(nc) as tc:
        with (
            tc.tile_pool(name="wpool", bufs=1) as wpool,
            tc.tile_pool(name="stage", bufs=4) as stage,
            tc.tile_pool(name="xpool", bufs=1 + XLOOK) as xpool,
            tc.tile_pool(name="opool", bufs=2) as opool,
            tc.tile_pool(name="bpool", bufs=1) as bpool,
            tc.tile_pool(name="psum", bufs=4, space="PSUM") as psump,
        ):
            # resident bf16 weights: one tile per (pair, k) of [128, 1024]
            # (2 o-blocks), 64 x 2 KB = 128 KB/partition. Split per pair so
            # pair-1 materialization writes disjoint tiles from the ones
            # pair-0 matmuls read (no dep-tracker hazards possible).
            w_tiles = {
                (h, k): wpool.tile(
                    [P, 2 * OC], BF16, name=f"w_{h}_{k}", tag=f"w_{h}_{k}"
                )
                for h in range(NOC // 2)
                for k in range(KT)
            }

            bias_bc = bpool.tile([P, O_PER], BF16, name="bias_bc")

            def softplus_fma(dst, rho_src, eps_src, mu_src, exp_t, sp_t):
                # dst = mu + softplus(rho) * eps, via Ln(Exp(rho) + 1).
                # exp/ln on scalar; mul/add on vector. During pair 0 the
                # compute half runs 2 m-chunks after its DMA was issued, so
                # its inputs are long since resident and the vector queue
                # (which also runs the latency-critical PSUM drains) never
                # blocks on a param DMA (v2/v4 lesson).
                nc.scalar.activation(exp_t, rho_src, AF.Exp)
                nc.scalar.activation(sp_t, exp_t, AF.Ln, bias=1.0)
                nc.vector.tensor_mul(sp_t, sp_t, eps_src)
                nc.vector.tensor_add(dst, sp_t, mu_src)

            def stage_tiles():
                prm = stage.tile([P, 3 * OCS], BF16, name="prm_s", tag="prm_s")
                exp_s = stage.tile([P, OCS], BF16, name="exp_s", tag="exp_s")
                sp_s = stage.tile([P, OCS], BF16, name="sp_s", tag="sp_s")
                return prm, exp_s, sp_s

            def materialize_dma(h, k, q):
                prm, exp_s, sp_s = stage_tiles()
                nc.sync.dma_start(prm, wprm[k, h, q, :, :])
                return (h, k, q, prm, exp_s, sp_s)

            def materialize_compute(st):
                h, k, q, prm, exp_s, sp_s = st
                softplus_fma(
                    w_tiles[(h, k)][:, bass.ts(q, OCS)],
                    prm[:, 0:OCS], prm[:, OCS : 2 * OCS], prm[:, 2 * OCS :],
                    exp_s, sp_s,
                )

            def materialize(h, k, q):
                materialize_compute(materialize_dma(h, k, q))

            # ── x slabs on the gpsimd queue, which does nothing else: a
            # slab issue can never be delayed by another engine's work.
            # Pair-0 issues all go up front (the WAR wait on each rotating
            # buffer self-paces the queue XLOOK slabs ahead); pair-1 issues
            # are emitted inside the pair-1 loop with the same lookahead.
            def x_load(mc):
                xt = xpool.tile([P, KT, M_CHUNK], BF16, name="xt", tag="xt")
                nc.gpsimd.dma_start(xt, xA[:, mc, :, :])
                return xt

            xts0 = [x_load(mc) for mc in range(MC)]

            # ── bias: softplus fma on partition 0, then replicate to all
            # 128 partitions via a doubling SBUF->SBUF DMA ladder (the
            # InstPartitionBroadcast custom op fails codegen here).
            for oc in range(O_PER // OCS):
                sl = bass.ts(oc, OCS)
                prm, exp_s, sp_s = stage_tiles()
                nc.sync.dma_start(prm[0:1, 0:OCS], brho[0:1, sl])
                nc.sync.dma_start(prm[0:1, OCS : 2 * OCS], beps[0:1, sl])
                nc.sync.dma_start(prm[0:1, 2 * OCS :], bmu[0:1, sl])
                softplus_fma(
                    bias_bc[0:1, sl], prm[0:1, 0:OCS], prm[0:1, OCS : 2 * OCS],
                    prm[0:1, 2 * OCS :], exp_s[0:1, :], sp_s[0:1, :],
                )
            rep = 1
            while rep < P:
                nc.sync.dma_start(bias_bc[rep : 2 * rep, :], bias_bc[0:rep, :])
                rep *= 2

            # ── blocks 0 and 1 (cols 0:1024) up front. q=0 (block 0) for
            # every k first: the first mm group needs only the q=0 chunks,
            # so it can chase the materialization wave with half the DMA.
            for q in range(2):
                for k in range(KT):
                    materialize(0, k, q)

            def mm_group(xt, j, mc):
                h, jj = j // 2, j % 2
                jsl = bass.ts(j, OC)
                ps = psump.tile([P, MSUB * OC], F32, name="ps", tag="ps")
                for k in range(KT):
                    for s in range(MSUB):
                        nc.tensor.matmul(
                            ps[:, bass.ts(s, OC)],
                            xt[:, k, bass.ts(s, P)],
                            w_tiles[(h, k)][:, bass.ts(jj, OC)],
                            start=(k == 0),
                            stop=(k == KT - 1),
                        )
                for s in range(MSUB):
                    out_sb = opool.tile([P, OC], F32, name="out_sb", tag="out_sb")
                    nc.vector.scalar_tensor_tensor(
                        out_sb,
                        ps[:, bass.ts(s, OC)],
                        1.0,
                        bias_bc[:, jsl],
                        op0=ALU.bypass,
                        op1=ALU.add,
                    )
                    # store issued from sync (vector/tensor can't issue
                    # DMAs; gpsimd is reserved for the x stream; scalar
                    # must stay free for exp/ln)
                    nc.sync.dma_start(
                        y[mc * M_CHUNK + s * P : mc * M_CHUNK + (s + 1) * P, jsl],
                        out_sb,
                    )

            # ── pair loop: pair 0 = blocks {0,1} (blocks 2,3 materialize
            # interleaved), pair 1 = blocks {2,3}; x read twice
            # pair 0: mm on blocks {0,1}; pair-1 params stream in (DMA at
            # mc, softplus at mc+1 so the vector ops never wait on the DMA)
            pending = []
            xt_q = {}
            for pair in range(NOC // 2):
                if pair == 1:
                    for m in range(min(XLOOK, MC)):
                        xt_q[m] = x_load(m)
                for mc in range(MC):
                    xt = xts0[mc] if pair == 0 else xt_q.pop(mc)
                    if pair == 1 and mc + XLOOK < MC:
                        xt_q[mc + XLOOK] = x_load(mc + XLOOK)
                    for dj in range(2):
                        mm_group(xt, 2 * pair + dj, mc)
                    if pair == 0:
                        pending.append(materialize_dma(1, mc, 0))
                        pending.append(materialize_dma(1, mc, 1))
                        while len(pending) > 2:
                            materialize_compute(pending.pop(0))
                if pair == 0:
                    while pending:
                        materialize_compute(pending.pop(0))

    _split_sync_waits(nc)
    nc.finalize()
    return nc


_NC_CACHE = None


def _get_nc():
    global _NC_CACHE
    if _NC_CACHE is None:
        _NC_CACHE = _build()
    return _NC_CACHE


def prepare_in_maps(x, weight_mu, weight_rho, weight_eps, bias_mu, bias_rho, bias_eps):
    x = np.asarray(x, dtype=np.float32)
    weight_mu = np.asarray(weight_mu, dtype=np.float32)
    weight_rho = np.asarray(weight_rho, dtype=np.float32)
    weight_eps = np.asarray(weight_eps, dtype=np.float32)
    bias_mu = np.asarray(bias_mu, dtype=np.float32)
    bias_rho = np.asarray(bias_rho, dtype=np.float32)
    bias_eps = np.asarray(bias_eps, dtype=np.float32)

    # xA[p, mc, kt, nn] = x[mc*M_CHUNK + nn, kt*P + p]
    xA = np.ascontiguousarray(
        x.reshape(MC, M_CHUNK, KT, P).transpose(3, 0, 2, 1)
    ).astype(ml_dtypes.bfloat16)

    def wprep(w, osl):
        # [kt, h, q, p, ocs]: wT chunk per (k-tile, pair-half, quarter)
        return (
            w[osl, :].T.reshape(KT, P, 2, 2, OCS).transpose(0, 2, 3, 1, 4)
        )

    in_maps = []
    for c in range(N_CORES):
        osl = slice(c * O_PER, (c + 1) * O_PER)
        # wprm[k, h, q, p, 3*OCS] = [rho | eps | mu] rows, 3 KB contiguous
        wprm = np.ascontiguousarray(
            np.concatenate(
                [
                    wprep(weight_rho, osl),
                    wprep(weight_eps, osl),
                    wprep(weight_mu, osl),
                ],
                axis=4,
            )
        ).astype(ml_dtypes.bfloat16)
        in_maps.append(
            {
                "xA": xA,
                "wprm": wprm,
                "bmu": bias_mu[osl].reshape(1, O_PER).astype(ml_dtypes.bfloat16),
                "brho": bias_rho[osl].reshape(1, O_PER).astype(ml_dtypes.bfloat16),
                "beps": bias_eps[osl].reshape(1, O_PER).astype(ml_dtypes.bfloat16),
            }
        )
    return in_maps


def run(in_maps, trace=False):
    nc = _get_nc()
    res = run_bass_kernel_spmd(nc, in_maps, list(range(N_CORES)), trace=trace)
    out = np.concatenate([res.results[c]["y"] for c in range(N_CORES)], axis=1)
    return out, res


def kernel(**inputs) -> np.ndarray:
    in_maps = prepare_in_maps(**inputs)
    out, _ = run(in_maps, trace=False)
    return out


# revision 47
# speedup vs baseline: 1.0123x; 1.0123x over previous
"""Bayesian linear layer on 8 Trainium2 NeuronCores.

Computes: weight = mu + softplus(rho) * eps  (elementwise, [O, I])
          bias   = b_mu + softplus(b_rho) * b_eps              ([O])
          y      = x @ weight.T + bias       ([N, I] @ [I, O] -> [N, O])

Shapes: x [8192, 4096], weight_* [16384, 4096], bias_* [16384].

Sharding: column-parallel over 8 cores - each core owns 2048 output
features (its slice of the weight/bias params), x is replicated. Each
core computes an independent [8192, 2048] output slice; the host
concatenates along the feature dim. No collectives needed.

Schedule notes. The kernel is PE-bound: 8192 bf16 matmuls of
[128,128]x[128,512] stream at ~220 ns each -> ~1.81 ms floor (measured
1.886 ms, PE ~93% busy). fp8-e4m3 DoubleRow measures the same ns/col
on this hw (2x MACs/cycle, not the cost model's 4x), so the 3-term
hi/lo fp8 split needed to pass the 2e-2 gate would cost 1.5x bf16 -
bf16 single pass is the fastest correct schedule. What keeps the PE
fed (each item fixed a measured stall class):
 - x host-repacked to [128, mc, kt, 256]: each m-chunk's load is one
   16 KB contiguous slab per partition (512-byte gathers ran the DMA
   engines at ~19 GB/s and starved the PE at chunk boundaries).
 - The gpsimd queue issues ONLY x-slab loads (pair-0's 32 up front;
   the pool-rotation WAR sems self-pace it XLOOK ahead). Pair-1 loads
   are emitted inside the pair-1 loop - emitting them earlier would
   deadlock: their buffer-WAR waits on pair-1 matmul progress would
   sit ahead of the pair-1 weight materialization in queue order.
 - The vector queue runs ONLY PSUM drains (+bias add) and nothing
   that can wait on a DMA: in-order queues transmit any materialize
   stall into late drains -> PSUM WAR -> PE stall (the dominant v2/v4
   failure mode).
 - Pair-1 weight materialization is split: the batched param DMA (one
   3 KB-row transfer, [rho|eps|mu]) issues on sync at m-chunk mc, its
   softplus (scalar exp/ln + vector mul/add) runs at mc+1, so the
   vector ops find their inputs resident.
 - y stores issue from sync; weights stay resident in SBUF across both
   pair passes (x read twice; blocks 2,3 materialize during pair 0).
 - Residual ~75 us: on-demand 16 KB instruction-page fetches (~1 us
   every ~25 us of tensor-queue program) and the startup chase (first
   block's softplus is scalar-engine-bound at ~46 us vs 14 us of PE
   demand).
"""

import numpy as np
import ml_dtypes

import concourse.bass as bass
import concourse.mybir as mybir
import concourse.tile as tile
from concourse.bass_utils import run_bass_kernel_spmd
from concourse.vector_clock import ScopedClock, VectorClock

N_CORES = 8
N_TOK = 8192
IN_F = 4096
OUT_F = 16384
O_PER = OUT_F // N_CORES  # 2048 out features per core

P = 128
KT = IN_F // P       # 32 k-tiles
OC = 512             # o-chunk: matmul moving width / PSUM bank
NOC = O_PER // OC    # 4 o-chunks

M_CHUNK = 256            # tokens per x slab (2 lhsT subtiles of 128)
MC = N_TOK // M_CHUNK    # 32 m-chunks
MSUB = M_CHUNK // P      # 2
OCS = 512                # stage chunk for weight materialization
XLOOK = 2                # x slabs enqueued ahead of consumption

F32 = mybir.dt.float32
BF16 = mybir.dt.bfloat16
AF = mybir.ActivationFunctionType
ALU = mybir.AluOpType


def _patch_tile_drain():
    """The walrus build here caps sync-wait commands per CTRL_NO_STRUCT
    instruction; Tile's kernel-tail Drain overflows it. Spread the waits
    across nop carriers (one wait each) before the drain."""
    if getattr(tile.TileContext, "_drain_patched", False):
        return

    def _drain_and_barrier(self, tick_clock, wait_clock):
        nc = self.nc
        gc = tick_clock.global_clock
        n = len(gc)
        for i in range(n):
            t = gc[i]
            if t > 0:
                sub = [0] * n
                sub[i] = t
                carrier = nc.sync.nop(nofuse=True)
                wait_clock.add_sem_waits(
                    carrier.ins, ScopedClock({None: VectorClock(sub)})
                )
        nc.sync.drain()
        nc.all_engine_barrier()
        popped = nc._tile_sem_poison_stack.pop()
        assert popped is self._sem_poison
        nc.clear_and_free_semaphores(list(self.sems.allocated().values()))
        nc.all_engine_barrier()

    tile.TileContext._drain_and_barrier = _drain_and_barrier
    tile.TileContext._drain_patched = True


def _split_sync_waits(nc, max_waits=1):
    """This container's walrus build accepts at most ONE sync-wait command
    per instruction (a 2-wait TensorTensor fails codegen with 'Too many
    sync wait commands'). Tile emits up to 3. Spill the excess onto
    same-engine InstNoOp carriers inserted immediately before the
    overloaded instruction - same-engine program order preserves the
    wait-before-execute semantics."""
    n_spilled = 0
    for fn in nc.m.functions:
        for bb in fn.blocks:
            insts = list(bb.instructions)
            out = []
            changed = False
            for inst in insts:
                si = inst.sync_info
                if si is not None and si.on_wait and len(si.on_wait) > max_waits:
                    waits = list(si.on_wait)
                    spill, keep = waits[:-max_waits], waits[-max_waits:]
                    for w in spill:
                        nop = mybir.InstNoOp(
                            name=f"I-waitspill-{nc.next_id()}", ins=[], outs=[]
                        )
                        nop.engine = inst.engine
                        nop.sync_info = mybir.SyncInfo(on_wait=[w], on_update=[])
                        out.append(nop)
                        n_spilled += 1
                    inst.sync_info = mybir.SyncInfo(
                        on_wait=keep, on_update=list(si.on_update)
                    )
                    changed = True
                out.append(inst)
            if changed:
                bb.instructions = out
    return n_spilled


def _build():
    _patch_tile_drain()
    nc = bass.Bass()

    # x: [p, mc, kt, n] so each (mc) slab is 16 KB contiguous per partition
    xA = nc.dram_tensor("xA", [P, MC, KT, M_CHUNK], BF16, kind="ExternalInput")
    # w params batched per stage set: [k, h, q, p, rho|eps|mu] - one 3 KB-row
    # DMA materializes a whole [128, OCS] weight chunk (3 issues -> 1)
    wprm = nc.dram_tensor(
        "wprm", [KT, 2, 2, P, 3 * OCS], BF16, kind="ExternalInput"
    )
    bmu = nc.dram_tensor("bmu", [1, O_PER], BF16, kind="ExternalInput")
    brho = nc.dram_tensor("brho", [1, O_PER], BF16, kind="ExternalInput")
    beps = nc.dram_tensor("beps", [1, O_PER], BF16, kind="ExternalInput")
    # y in bf16: halves store traffic (67 -> 33.5 MB/core); rounding adds
    # ~1e-3 absmax-rel on top of 4.1e-3 against the 2e-2 gate
    y = nc.dram_tensor("y", [N_TOK, O_PER], BF16, kind="ExternalOutput")

    with tile.TileContext(nc) as tc:
        with (
            tc.tile_pool(name="wpool", bufs=1) as wpool,
            tc.tile_pool(name="stage", bufs=4) as stage,
            tc.tile_pool(name="xpool", bufs=1 + XLOOK) as xpool,
            tc.tile_pool(name="opool", bufs=2) as opool,
            tc.tile_pool(name="bpool", bufs=1) as bpool,
            tc.tile_pool(name="psum", bufs=4, space="PSUM") as psump,
        ):
            # resident bf16 weights: one tile per (pair, k) of [128, 1024]
            # (2 o-blocks), 64 x 2 KB = 128 KB/partition. Split per pair so
            # pair-1 materialization writes disjoint tiles from the ones
            # pair-0 matmuls read (no dep-tracker hazards possible).
            w_tiles = {
                (h, k): wpool.tile(
                    [P, 2 * OC], BF16, name=f"w_{h}_{k}", tag=f"w_{h}_{k}"
                )
                for h in range(NOC // 2)
                for k in range(KT)
            }

            bias_bc = bpool.tile([P, O_PER], BF16, name="bias_bc")

            def softplus_fma(dst, rho_src, eps_src, mu_src, exp_t, sp_t):
                # dst = mu + softplus(rho) * eps, via Ln(Exp(rho) + 1).
                # exp/ln on scalar; mul/add on vector. During pair 0 the
                # compute half runs 2 m-chunks after its DMA was issued, so
                # its inputs are long since resident and the vector queue
                # (which also runs the latency-critical PSUM drains) never
                # blocks on a param DMA (v2/v4 lesson).
                nc.scalar.activation(exp_t, rho_src, AF.Exp)
                nc.scalar.activation(sp_t, exp_t, AF.Ln, bias=1.0)
                nc.vector.tensor_mul(sp_t, sp_t, eps_src)
                nc.vector.tensor_add(dst, sp_t, mu_src)

            def stage_tiles():
                prm = stage.tile([P, 3 * OCS], BF16, name="prm_s", tag="prm_s")
                exp_s = stage.tile([P, OCS], BF16, name="exp_s", tag="exp_s")
                sp_s = stage.tile([P, OCS], BF16, name="sp_s", tag="sp_s")
                return prm, exp_s, sp_s

            def materialize_dma(h, k, q):
                prm, exp_s, sp_s = stage_tiles()
                nc.sync.dma_start(prm, wprm[k, h, q, :, :])
                return (h, k, q, prm, exp_s, sp_s)

            def materialize_compute(st):
                h, k, q, prm, exp_s, sp_s = st
                softplus_fma(
                    w_tiles[(h, k)][:, bass.ts(q, OCS)],
                    prm[:, 0:OCS], prm[:, OCS : 2 * OCS], prm[:, 2 * OCS :],
                    exp_s, sp_s,
                )

            def materialize(h, k, q):
                materialize_compute(materialize_dma(h, k, q))

            # ── x slabs on the gpsimd queue, which does nothing else: a
            # slab issue can never be delayed by another engine's work.
            # Pair-0 issues all go up front (the WAR wait on each rotating
            # buffer self-paces the queue XLOOK slabs ahead); pair-1 issues
            # are emitted inside the pair-1 loop with the same lookahead.
            def x_load(mc):
                xt = xpool.tile([P, KT, M_CHUNK], BF16, name="xt", tag="xt")
                nc.gpsimd.dma_start(xt, xA[:, mc, :, :])
                return xt

            xts0 = [x_load(mc) for mc in range(MC)]

            # ── bias: softplus fma on partition 0, then replicate to all
            # 128 partitions via a doubling SBUF->SBUF DMA ladder (the
            # InstPartitionBroadcast custom op fails codegen here).
            for oc in range(O_PER // OCS):
                sl = bass.ts(oc, OCS)
                prm, exp_s, sp_s = stage_tiles()
                nc.sync.dma_start(prm[0:1, 0:OCS], brho[0:1, sl])
                nc.sync.dma_start(prm[0:1, OCS : 2 * OCS], beps[0:1, sl])
                nc.sync.dma_start(prm[0:1, 2 * OCS :], bmu[0:1, sl])
                softplus_fma(
                    bias_bc[0:1, sl], prm[0:1, 0:OCS], prm[0:1, OCS : 2 * OCS],
                    prm[0:1, 2 * OCS :], exp_s[0:1, :], sp_s[0:1, :],
                )
            rep = 1
            while rep < P:
                nc.sync.dma_start(bias_bc[rep : 2 * rep, :], bias_bc[0:rep, :])
                rep *= 2

            # ── blocks 0 and 1 (cols 0:1024) up front. q=0 (block 0) for
            # every k first: the first mm group needs only the q=0 chunks,
            # so it can chase the materialization wave with half the DMA.
            for q in range(2):
                for k in range(KT):
                    materialize(0, k, q)

            def mm_group(xt, j, mc):
                h, jj = j // 2, j % 2
                jsl = bass.ts(j, OC)
                ps = psump.tile([P, MSUB * OC], F32, name="ps", tag="ps")
                for k in range(KT):
                    for s in range(MSUB):
                        nc.tensor.matmul(
                            ps[:, bass.ts(s, OC)],
                            xt[:, k, bass.ts(s, P)],
                            w_tiles[(h, k)][:, bass.ts(jj, OC)],
                            start=(k == 0),
                            stop=(k == KT - 1),
                        )
                for s in range(MSUB):
                    out_sb = opool.tile([P, OC], BF16, name="out_sb", tag="out_sb")
                    nc.vector.scalar_tensor_tensor(
                        out_sb,
                        ps[:, bass.ts(s, OC)],
                        1.0,
                        bias_bc[:, jsl],
                        op0=ALU.bypass,
                        op1=ALU.add,
                    )
                    # store issued from sync (vector/tensor can't issue
                    # DMAs; gpsimd is reserved for the x stream; scalar
                    # must stay free for exp/ln)
                    nc.sync.dma_start(
                        y[mc * M_CHUNK + s * P : mc * M_CHUNK + (s + 1) * P, jsl],
                        out_sb,
                    )

            # ── pair loop: pair 0 = blocks {0,1} (blocks 2,3 materialize
            # interleaved), pair 1 = blocks {2,3}; x read twice
            # pair 0: mm on blocks {0,1}; pair-1 params stream in (DMA at
            # mc, softplus at mc+1 so the vector ops never wait on the DMA)
            pending = []
            xt_q = {}
            for pair in range(NOC // 2):
                if pair == 1:
                    for m in range(min(XLOOK, MC)):
                        xt_q[m] = x_load(m)
                for mc in range(MC):
                    xt = xts0[mc] if pair == 0 else xt_q.pop(mc)
                    if pair == 1 and mc + XLOOK < MC:
                        xt_q[mc + XLOOK] = x_load(mc + XLOOK)
                    for dj in range(2):
                        mm_group(xt, 2 * pair + dj, mc)
                    if pair == 0:
                        pending.append(materialize_dma(1, mc, 0))
                        pending.append(materialize_dma(1, mc, 1))
                        while len(pending) > 2:
                            materialize_compute(pending.pop(0))
                if pair == 0:
                    while pending:
                        materialize_compute(pending.pop(0))

    _split_sync_waits(nc)
    nc.finalize()
    return nc


_NC_CACHE = None


def _get_nc():
    global _NC_CACHE
    if _NC_CACHE is None:
        _NC_CACHE = _build()
    return _NC_CACHE


def prepare_in_maps(x, weight_mu, weight_rho, weight_eps, bias_mu, bias_rho, bias_eps):
    x = np.asarray(x, dtype=np.float32)
    weight_mu = np.asarray(weight_mu, dtype=np.float32)
    weight_rho = np.asarray(weight_rho, dtype=np.float32)
    weight_eps = np.asarray(weight_eps, dtype=np.float32)
    bias_mu = np.asarray(bias_mu, dtype=np.float32)
    bias_rho = np.asarray(bias_rho, dtype=np.float32)
    bias_eps = np.asarray(bias_eps, dtype=np.float32)

    # xA[p, mc, kt, nn] = x[mc*M_CHUNK + nn, kt*P + p]
    xA = np.ascontiguousarray(
        x.reshape(MC, M_CHUNK, KT, P).transpose(3, 0, 2, 1)
    ).astype(ml_dtypes.bfloat16)

    def wprep(w, osl):
        # [kt, h, q, p, ocs]: wT chunk per (k-tile, pair-half, quarter)
        return (
            w[osl, :].T.reshape(KT, P, 2, 2, OCS).transpose(0, 2, 3, 1, 4)
        )

    in_maps = []
    for c in range(N_CORES):
        osl = slice(c * O_PER, (c + 1) * O_PER)
        # wprm[k, h, q, p, 3*OCS] = [rho | eps | mu] rows, 3 KB contiguous
        wprm = np.ascontiguousarray(
            np.concatenate(
                [
                    wprep(weight_rho, osl),
                    wprep(weight_eps, osl),
                    wprep(weight_mu, osl),
                ],
                axis=4,
            )
        ).astype(ml_dtypes.bfloat16)
        in_maps.append(
            {
                "xA": xA,
                "wprm": wprm,
                "bmu": bias_mu[osl].reshape(1, O_PER).astype(ml_dtypes.bfloat16),
                "brho": bias_rho[osl].reshape(1, O_PER).astype(ml_dtypes.bfloat16),
                "beps": bias_eps[osl].reshape(1, O_PER).astype(ml_dtypes.bfloat16),
            }
        )
    return in_maps


def run(in_maps, trace=False):
    nc = _get_nc()
    res = run_bass_kernel_spmd(nc, in_maps, list(range(N_CORES)), trace=trace)
    out = np.concatenate(
        [res.results[c]["y"] for c in range(N_CORES)], axis=1
    ).astype(np.float32)
    return out, res


def kernel(**inputs) -> np.ndarray:
    in_maps = prepare_in_maps(**inputs)
    out, _ = run(in_maps, trace=False)
    return out
